# revision 9
# baseline (speedup 1.0000x reference)
"""Trainium2 Bass kernel: segmented (expert-parallel) LoRA with dropout.

Computes  out = result + scatter_e( (data_e * keep_e * scale) @ A_e^T @ B_e^T )
where keep = (drop_mask >= 0.05), scale = 2.0 / 0.95, and each of the E=8
adapters owns a contiguous batch segment of 2 batches (4096 tokens).

Sharding: expert-parallel — core e gets adapter e's A/B and its batch segment
(data/drop_mask/result slices), so there are no cross-core collectives.

Layout: the host hands each core its slices TRANSPOSED to [H, tok] (hidden on
partitions). The PE contracts over the partition dim, so LoRA's rank-16 GEMMs
need hidden-major operands; transposing on the host keeps the TensorEngine
free of the 1024 transpose+ldweights pairs per core that otherwise dominate
(measured 2/3 of PE time and they keep the PE clock-gate cold).

Per-core dataflow, per token half (2048 tokens, pipelined across halves):
  Phase 1, per 128-wide h chunk (32 chunks):
    - DMA in dataT/maskT chunk [128, 2048] fp32.
    - DVE fused dropout: dropped = (mask >= 0.05) * data -> bf16 (scale is
      folded into A on the host).
    - GEMM1: 4 matmuls (N=512) accumulate midT[r=16, tok] into 4 PSUM banks
      held across the whole h loop (full contraction over H).
  - ACT copies midT PSUM -> SBUF bf16.
  Phase 2, per h chunk:
    - DMA in resT chunk [128, 2048] fp32.
    - GEMM2: 4 matmuls outT_psum[128 h, 512 tok] = B_chunk^T @ midT.
    - DVE tensor_add: outT = outT_psum + resT -> SBUF, DMA out.
Phase 2 of half A overlaps phase 1 of half B (disjoint PSUM banks: 4+4).

Weights are host-packed into the exact SBUF layouts (tiny: 128 KB each):
  a_pk[p, c*R+j] = A[j, c*128+p] * scale   (bf16)  == scaled A^T chunks
  b_pk[j, h]     = B[h, j]                 (bf16)  == B^T
"""

import numpy as np
from contextlib import ExitStack

import ml_dtypes

from concourse import bass, bacc, mybir, tile
from concourse.bass_utils import run_bass_kernel_spmd

# Pin all HWDGE DMA completions to a single semaphore lane (DMAHW0). HWDGE
# DMAs issued from one engine complete in FIFO order anyway, so merging the
# lanes loses nothing, and consumers then need a single DMA sync-wait instead
# of one per lane (each extra wait costs a split event-semaphore instruction).
import concourse.tile_sem_assignment as _tsa
_tsa.NUM_HWDGE_SEMS = 1

# Problem constants (hardcoded per the self-contained-kernel contract).
E = 8
B, S, H, R = 16, 2048, 4096, 16
SEG = B // E
TOK = SEG * S          # tokens per core = 4096
P = 128                # partitions
P_DROP = 0.05
SCALING = 2.0
SCALE = SCALING / (1.0 - P_DROP)

F32 = mybir.dt.float32
BF16 = mybir.dt.bfloat16
BF16_NP = ml_dtypes.bfloat16

LAST_RESULTS = None    # BassKernelResults of the most recent run (for test.py)


def build_nc(tok=TOK, h=H, r=R, num_devices=E, halves=2):
    """Build the single-core Bass/Tile program (run SPMD on all cores)."""
    hc = h // P                    # 128-wide h chunks
    thk = tok // halves            # tokens per half
    tb = thk // 512                # 512-wide token blocks per half
    assert tok % halves == 0 and thk % 512 == 0 and h % P == 0

    nc = bacc.Bacc("TRN2", target_bir_lowering=False, debug=False,
                   num_devices=num_devices)

    data = nc.dram_tensor("data", [h, tok], F32, kind="ExternalInput").ap()
    mask = nc.dram_tensor("mask", [h, tok], F32, kind="ExternalInput").ap()
    res = nc.dram_tensor("res", [h, tok], F32, kind="ExternalInput").ap()
    a_pk = nc.dram_tensor("a_pk", [P, hc * r], BF16, kind="ExternalInput").ap()
    b_pk = nc.dram_tensor("b_pk", [r, h], BF16, kind="ExternalInput").ap()
    out = nc.dram_tensor("out", [h, tok], F32, kind="ExternalOutput").ap()

    with ExitStack() as ctx:
        tc = ctx.enter_context(tile.TileContext(nc))
        consts = ctx.enter_context(tc.tile_pool(name="consts", bufs=1))
        loads = ctx.enter_context(tc.tile_pool(name="loads", bufs=3))
        work = ctx.enter_context(tc.tile_pool(name="work", bufs=3))
        outp = ctx.enter_context(tc.tile_pool(name="outp", bufs=3))
        # PSUM budget (8 banks): midT [16, thk] f32 = thk/512 banks (bufs=1,
        # freed by the ACT copy at the phase boundary) + pso 4 x 1 bank.
        psm = ctx.enter_context(tc.tile_pool(name="psm", bufs=1, space="PSUM"))
        pso = ctx.enter_context(tc.tile_pool(name="pso", bufs=4, space="PSUM"))

        a_sb = consts.tile([P, hc * r], BF16)
        nc.sync.dma_start(a_sb, a_pk)
        b_sb = consts.tile([r, h], BF16)
        nc.sync.dma_start(b_sb, b_pk)

        for hf in range(halves):
            tcols = bass.ts(hf, thk)

            # -- phase 1: dropout + GEMM1, accumulating midT in PSUM --------
            midT_ps = psm.tile([r, thk], F32)   # tb banks, live across h loop
            for c in range(hc):
                hrows = bass.ts(c, P)
                mask_sb = loads.tile([P, thk], F32)
                nc.sync.dma_start(mask_sb, mask[hrows, tcols])
                data_sb = loads.tile([P, thk], F32)
                nc.sync.dma_start(data_sb, data[hrows, tcols])

                # dropped = (mask >= p) * data, bf16 (scale folded into A)
                drop_sb = work.tile([P, thk], BF16)
                nc.vector.scalar_tensor_tensor(
                    drop_sb, mask_sb, P_DROP, data_sb,
                    op0=mybir.AluOpType.is_ge, op1=mybir.AluOpType.mult)

                for t in range(tb):
                    nc.tensor.matmul(
                        midT_ps[:, bass.ts(t, 512)],
                        lhsT=a_sb[:, bass.ts(c, r)],
                        rhs=drop_sb[:, bass.ts(t, 512)],
                        start=(c == 0), stop=(c == hc - 1))

            midT_sb = work.tile([r, thk], BF16)
            nc.scalar.copy(midT_sb, midT_ps)

            # -- phase 2: GEMM2 + residual add + store ----------------------
            for c in range(hc):
                hrows = bass.ts(c, P)
                res_sb = loads.tile([P, thk], F32)
                nc.sync.dma_start(res_sb, res[hrows, tcols])

                outT_sb = outp.tile([P, thk], F32)
                for t in range(tb):
                    o_ps = pso.tile([P, 512], F32)
                    nc.tensor.matmul(
                        o_ps, lhsT=b_sb[:, hrows],
                        rhs=midT_sb[:, bass.ts(t, 512)],
                        start=True, stop=True)
                    nc.vector.tensor_add(outT_sb[:, bass.ts(t, 512)], o_ps,
                                         res_sb[:, bass.ts(t, 512)])
                nc.sync.dma_start(out[hrows, tcols], outT_sb)
    nc.compile()
    return nc


def pack_weights(lora_a, lora_b, h=H, r=R):
    """Pack A (pre-scaled) and B into the SBUF layouts the kernel expects."""
    e = lora_a.shape[0]
    hc = h // P
    a_sc = (np.asarray(lora_a, np.float32) * SCALE).astype(BF16_NP)   # (E,R,H)
    a_pk = np.ascontiguousarray(
        a_sc.reshape(e, r, hc, P).transpose(0, 3, 2, 1)).reshape(e, P, hc * r)
    b_pk = np.ascontiguousarray(
        np.asarray(lora_b, np.float32).astype(BF16_NP).transpose(0, 2, 1))
    return a_pk, b_pk


def kernel(result, data, drop_mask, lora_a, lora_b, _trace=False):
    global LAST_RESULTS
    result = np.asarray(result, np.float32)
    data = np.asarray(data, np.float32)
    drop_mask = np.asarray(drop_mask, np.float32)

    # per-core slices, transposed to [H, tok] (hidden-major for the PE)
    data_t = np.ascontiguousarray(
        data.reshape(E, TOK, H).transpose(0, 2, 1))
    mask_t = np.ascontiguousarray(
        drop_mask.reshape(E, TOK, H).transpose(0, 2, 1))
    res_t = np.ascontiguousarray(
        result.reshape(E, TOK, H).transpose(0, 2, 1))
    a_pk, b_pk = pack_weights(lora_a, lora_b)

    nc = build_nc()
    in_maps = [
        {"data": data_t[e], "mask": mask_t[e], "res": res_t[e],
         "a_pk": a_pk[e], "b_pk": b_pk[e]}
        for e in range(E)
    ]
    LAST_RESULTS = run_bass_kernel_spmd(
        nc, in_maps, core_ids=list(range(E)), trace=_trace)
    out_t = np.stack([LAST_RESULTS.results[e]["out"] for e in range(E)])
    return np.ascontiguousarray(out_t.transpose(0, 2, 1)).reshape(B, S, H)


if __name__ == "__main__":
    rng = np.random.default_rng(0)
    inputs = {
        "result": rng.standard_normal((B, S, H), dtype=np.float32),
        "data": rng.standard_normal((B, S, H), dtype=np.float32),
        "drop_mask": rng.random((B, S, H), dtype=np.float32),
        "lora_a": (rng.standard_normal((E, R, H), dtype=np.float32) * 0.02),
        "lora_b": (rng.standard_normal((E, H, R), dtype=np.float32) * 0.02),
    }
    out = kernel(**inputs)
    print("out", out.shape, out.dtype)


# revision 10
# speedup vs baseline: 1.8551x; 1.8551x over previous
"""Trainium2 Bass kernel: segmented (expert-parallel) LoRA with dropout.

Computes  out = result + scatter_e( (data_e * keep_e * scale) @ A_e^T @ B_e^T )
where keep = (drop_mask >= 0.05), scale = 2.0 / 0.95, and each of the E=8
adapters owns a contiguous batch segment of 2 batches (4096 tokens).

Sharding: expert-parallel — core e gets adapter e's A/B and its batch segment
(data/drop_mask/result slices), so there are no cross-core collectives.

Layout: the host hands each core its slices TRANSPOSED to [H, tok] (hidden on
partitions). The PE contracts over the partition dim, so LoRA's rank-16 GEMMs
need hidden-major operands; transposing on the host keeps the TensorEngine
free of the 1024 transpose+ldweights pairs per core that otherwise dominate
(measured: 2/3 of PE time, and they keep the PE clock-gate cold).

DMA: the kernel is HBM-bound (256 MB/core at ~358 GB/s = ~715 us floor).
Measured: back-to-back DMAs on one descriptor ring serialize with ~2-3 us of
fixed latency each, so the work is split across TWO independent rings — SP
(HWDGE) carries data+result loads, GpSimd (SWDGE) carries mask loads and
output stores — with one completion-semaphore lane per ring (each ring is
FIFO, so one cumulative lane per ring is exact).

Per-core dataflow (all transfers are full-width [128, 4096] = 2 MB rows):
  Phase 1, per 128-wide h chunk (32 chunks):
    - DMA in dataT (SP) / maskT (SWDGE) chunk fp32.
    - DVE fused dropout: dropped = (mask >= 0.05) * data -> bf16 (scale is
      folded into A on the host).
    - GEMM1: 8 matmuls (N=512) accumulate midT[16, 4096] across the h loop
      in 8 PSUM banks (full contraction over H).
  - ACT copies midT PSUM -> SBUF bf16 (frees all 8 banks).
  Phase 2, per h chunk:
    - DMA in resT chunk (SP).
    - GEMM2: 8 matmuls outT_psum[128, 512] = B_chunk^T @ midT into 4-bank
      PSUM tiles (2 slots, double-buffered).
    - DVE tensor_add: outT = outT_psum + resT -> SBUF, DMA out (SWDGE).

Weights are host-packed into the exact SBUF layouts (tiny: 128 KB each):
  a_pk[p, c*R+j] = A[j, c*128+p] * scale   (bf16)  == scaled A^T chunks
  b_pk[j, h]     = B[h, j]                 (bf16)  == B^T
"""

import numpy as np
from contextlib import ExitStack

import ml_dtypes

from concourse import bass, bacc, mybir, tile
from concourse.bass_utils import run_bass_kernel_spmd

# One completion-semaphore lane per DMA ring. Each ring (SP HWDGE, GpSimd
# SWDGE) completes its DMAs in FIFO order, so a single cumulative lane per
# ring is exact — and consumers then need one DMA sync-wait per ring instead
# of one per round-robin lane (excess waits cost split event-semaphores).
import concourse.tile_sem_assignment as _tsa
_tsa.NUM_HWDGE_SEMS = 1
_tsa.NUM_SWDGE_GLOBAL_SEMS = 1

# Problem constants (hardcoded per the self-contained-kernel contract).
E = 8
B, S, H, R = 16, 2048, 4096, 16
SEG = B // E
TOK = SEG * S          # tokens per core = 4096
P = 128                # partitions
P_DROP = 0.05
SCALING = 2.0
SCALE = SCALING / (1.0 - P_DROP)

F32 = mybir.dt.float32
BF16 = mybir.dt.bfloat16
BF16_NP = ml_dtypes.bfloat16

LAST_RESULTS = None    # BassKernelResults of the most recent run (for test.py)


def build_nc(tok=TOK, h=H, r=R, num_devices=E):
    """Build the single-core Bass/Tile program (run SPMD on all cores)."""
    hc = h // P                    # 128-wide h chunks
    tb = tok // 512                # 512-wide token blocks
    tbh = tb // 2                  # token blocks per PSUM half
    assert h % P == 0 and tok % 1024 == 0

    nc = bacc.Bacc("TRN2", target_bir_lowering=False, debug=False,
                   num_devices=num_devices)

    data = nc.dram_tensor("data", [h, tok], F32, kind="ExternalInput").ap()
    mask = nc.dram_tensor("mask", [h, tok], F32, kind="ExternalInput").ap()
    res = nc.dram_tensor("res", [h, tok], F32, kind="ExternalInput").ap()
    a_pk = nc.dram_tensor("a_pk", [P, hc * r], BF16, kind="ExternalInput").ap()
    b_pk = nc.dram_tensor("b_pk", [r, h], BF16, kind="ExternalInput").ap()
    out = nc.dram_tensor("out", [h, tok], F32, kind="ExternalOutput").ap()

    with ExitStack() as ctx:
        tc = ctx.enter_context(tile.TileContext(nc))
        consts = ctx.enter_context(tc.tile_pool(name="consts", bufs=1))
        ld_d = ctx.enter_context(tc.tile_pool(name="ld_d", bufs=3))
        ld_m = ctx.enter_context(tc.tile_pool(name="ld_m", bufs=2))
        ld_r = ctx.enter_context(tc.tile_pool(name="ld_r", bufs=2))
        work = ctx.enter_context(tc.tile_pool(name="work", bufs=2))
        outp = ctx.enter_context(tc.tile_pool(name="outp", bufs=2))
        # One PSUM pool, 2 slots x 4 banks: phase 1 holds midT in both slots
        # ([16, tok/2] each); phase 2 double-buffers GEMM2 tiles [128, tok/2].
        ps = ctx.enter_context(tc.tile_pool(name="ps", bufs=2, space="PSUM"))

        a_sb = consts.tile([P, hc * r], BF16)
        nc.sync.dma_start(a_sb, a_pk)
        b_sb = consts.tile([r, h], BF16)
        nc.sync.dma_start(b_sb, b_pk)

        # -- phase 1: dropout + GEMM1, midT accumulates across the h loop ---
        mids = [ps.tile([r, tok // 2], F32, tag="ps", name=f"midT_{i}")
                for i in range(2)]
        for c in range(hc):
            hrows = bass.ts(c, P)
            mask_sb = ld_m.tile([P, tok], F32)
            nc.gpsimd.dma_start(mask_sb, mask[hrows, :])
            data_sb = ld_d.tile([P, tok], F32)
            nc.sync.dma_start(data_sb, data[hrows, :])

            # dropped = (mask >= p) * data, bf16 (scale folded into A)
            drop_sb = work.tile([P, tok], BF16)
            nc.vector.scalar_tensor_tensor(
                drop_sb, mask_sb, P_DROP, data_sb,
                op0=mybir.AluOpType.is_ge, op1=mybir.AluOpType.mult)

            for t in range(tb):
                nc.tensor.matmul(
                    mids[t // tbh][:, bass.ts(t % tbh, 512)],
                    lhsT=a_sb[:, bass.ts(c, r)],
                    rhs=drop_sb[:, bass.ts(t, 512)],
                    start=(c == 0), stop=(c == hc - 1))

        midT_sb = work.tile([r, tok], BF16)
        nc.scalar.copy(midT_sb[:, : tok // 2], mids[0])
        nc.scalar.copy(midT_sb[:, tok // 2:], mids[1])

        # -- phase 2: GEMM2 + residual add + store --------------------------
        for c in range(hc):
            hrows = bass.ts(c, P)
            res_sb = ld_r.tile([P, tok], F32)
            nc.sync.dma_start(res_sb, res[hrows, :])

            outT_sb = outp.tile([P, tok], F32)
            for half in range(2):
                tcols = bass.ts(half, tok // 2)
                o_ps = ps.tile([P, tok // 2], F32, tag="ps")
                for t in range(tbh):
                    nc.tensor.matmul(
                        o_ps[:, bass.ts(t, 512)], lhsT=b_sb[:, hrows],
                        rhs=midT_sb[:, bass.ts(half * tbh + t, 512)],
                        start=True, stop=True)
                nc.vector.tensor_add(outT_sb[:, tcols], o_ps, res_sb[:, tcols])
            nc.gpsimd.dma_start(out[hrows, :], outT_sb)
    nc.compile()
    return nc


def pack_weights(lora_a, lora_b, h=H, r=R):
    """Pack A (pre-scaled) and B into the SBUF layouts the kernel expects."""
    e = lora_a.shape[0]
    hc = h // P
    a_sc = (np.asarray(lora_a, np.float32) * SCALE).astype(BF16_NP)   # (E,R,H)
    a_pk = np.ascontiguousarray(
        a_sc.reshape(e, r, hc, P).transpose(0, 3, 2, 1)).reshape(e, P, hc * r)
    b_pk = np.ascontiguousarray(
        np.asarray(lora_b, np.float32).astype(BF16_NP).transpose(0, 2, 1))
    return a_pk, b_pk


def kernel(result, data, drop_mask, lora_a, lora_b, _trace=False):
    global LAST_RESULTS
    result = np.asarray(result, np.float32)
    data = np.asarray(data, np.float32)
    drop_mask = np.asarray(drop_mask, np.float32)

    # per-core slices, transposed to [H, tok] (hidden-major for the PE)
    data_t = np.ascontiguousarray(
        data.reshape(E, TOK, H).transpose(0, 2, 1))
    mask_t = np.ascontiguousarray(
        drop_mask.reshape(E, TOK, H).transpose(0, 2, 1))
    res_t = np.ascontiguousarray(
        result.reshape(E, TOK, H).transpose(0, 2, 1))
    a_pk, b_pk = pack_weights(lora_a, lora_b)

    nc = build_nc()
    in_maps = [
        {"data": data_t[e], "mask": mask_t[e], "res": res_t[e],
         "a_pk": a_pk[e], "b_pk": b_pk[e]}
        for e in range(E)
    ]
    LAST_RESULTS = run_bass_kernel_spmd(
        nc, in_maps, core_ids=list(range(E)), trace=_trace)
    out_t = np.stack([LAST_RESULTS.results[e]["out"] for e in range(E)])
    return np.ascontiguousarray(out_t.transpose(0, 2, 1)).reshape(B, S, H)


if __name__ == "__main__":
    rng = np.random.default_rng(0)
    inputs = {
        "result": rng.standard_normal((B, S, H), dtype=np.float32),
        "data": rng.standard_normal((B, S, H), dtype=np.float32),
        "drop_mask": rng.random((B, S, H), dtype=np.float32),
        "lora_a": (rng.standard_normal((E, R, H), dtype=np.float32) * 0.02),
        "lora_b": (rng.standard_normal((E, H, R), dtype=np.float32) * 0.02),
    }
    out = kernel(**inputs)
    print("out", out.shape, out.dtype)
